# revision 15
# baseline (speedup 1.0000x reference)
"""Trainium2 Bass kernel for nn_Composer (gnn_message_passing).

Math per block (DEPTH=2 blocks, same weights):
    tde[t,n]  = tanh( sum_{e,d} W1[t,d,e] * tok[d,n] * dep[e,n] + b1[t] )
    cnz[p,n]  = tanh( sum_{t,d} W2[p,d,t] * tok[d,n] * tde[t,n] + b2[p] )
    tok'[p,i] = base[p] + sum_j wr[j] * (cnz[p,j] - tanh(b2)[p]) * [heads[j]==i]
Final: out = tok * (heads == 0).

Device strategy (8 cores, data-parallel over batch, 2 batches/core, n=256):
  - feature-major layout [feature_partition, n_free]; bf16 pipe (fp32 psum)
  - bilinear contractions as PE matmuls over K-tiles with PSUM accumulation;
    moving operand z = tok * rep(second_factor) built by DVE tensor_tensor
  - the partition-replication of tde is spread across three engines so it
    never serializes on one resource, with a per-block route mix matched to
    when the DMA bus is free:
      * PE one-hot-selector matmul (out[p,n]=sum_k ident[k,j]tde[k,n]) with
        ACT-engine PSUM evacuation -- no DRAM round trip, finest pipelining
      * GPSIMD partition_broadcast from a single-partition SBUF row
      * DMA broadcast from a DRAM scratch copy
  - embeddings (token+dep) gathered on host, shipped in device layout
  - one-hot scatter matrices H built on host, wr folded in; for the final
    block the scatter matmul is emitted TRANSPOSED (lhsT=H, rhs=delta^T)
    so it directly produces the [token, feature] output layout, with the
    root mask and base vector folded in on host
"""

import os
import sys

sys.path.insert(0, "/opt/trn_rl_repo")

import ml_dtypes
import numpy as np

import concourse.bass as bass
import concourse.bacc as bacc
import concourse.mybir as mybir
import concourse.tile as tile
from concourse.bass_utils import run_bass_kernel_spmd

B, S, D, E, T = 16, 128, 128, 64, 128
V_TOK, V_DEP = 100000, 64
DEPTH = 2
NCORES = 8
BL = B // NCORES  # local batches per core
N = BL * S        # positions per core
F32 = mybir.dt.float32
I32 = mybir.dt.int32
BF16 = mybir.dt.bfloat16

CH_Z = 16  # dep-rep chunk size (j-tiles per chunk; 64 z-tiles total)
CH_X = 16  # tde-rep chunk size (128 x-tiles total)

# bilinear-2 replication route per 16-j chunk, per block:
#   'e' = PE selector-matmul, 'p' = gpsimd partition_broadcast, 'd' = DMA
REP_ROUTES = [
    ["p", "e", "e", "p", "d", "e", "d", "d"],  # block 0: DMA busy with W2
    ["e", "d", "d", "p", "d", "d", "p", "d"],  # block 1: DMA bus is free
]

# packed f32 constant layout (columns)
C_IDENT = 0          # [0,128)   identity
C_B1 = 128           # b1
C_B2 = 129           # b2
C_CBG = 130          # tanh(b2)
C_BASE = 131         # sum(wr)*tanh(b2)+br
C_BASEB = 132        # [132,132+N): outer(mask_b, base) per batch, [n,d] rows
C_TOT = 132 + N

LAST_EXEC_TIME_NS = None


def build_program():
    MV = BF16
    nc = bacc.Bacc("TRN2", target_bir_lowering=False, debug=False)
    w1t = nc.dram_tensor("W1t", [128, (E * D // 128) * T], MV, kind="ExternalInput")
    w2t = nc.dram_tensor("W2t", [128, (T * D // 128) * D], MV, kind="ExternalInput")
    cpackh = nc.dram_tensor("cpack", [128, C_TOT], F32, kind="ExternalInput")
    hh = nc.dram_tensor("Hpack", [128, DEPTH * BL * 128], MV, kind="ExternalInput")
    tok0h = nc.dram_tensor("tok0", [128, N], MV, kind="ExternalInput")
    deph = nc.dram_tensor("dep_flat", [1, E * N], MV, kind="ExternalInput")
    outh = nc.dram_tensor("out", [BL, S, D], F32, kind="ExternalOutput")

    NZ = E // CH_Z
    NX = T // CH_X

    with tile.TileContext(nc) as tc:
        with (
            tc.tile_pool(name="const", bufs=1) as cpool,
            tc.tile_pool(name="wres", bufs=1) as wpool,
            tc.tile_pool(name="zc", bufs=2) as zpool,
            tc.tile_pool(name="rept", bufs=3) as rtpool,
            tc.tile_pool(name="reprow", bufs=2) as rowpool,
            tc.tile_pool(name="xc", bufs=3) as xpool,
            tc.tile_pool(name="work", bufs=2) as work,
            tc.tile_pool(name="psmm", bufs=2, space="PSUM") as pspool,
            tc.tile_pool(name="pssm", bufs=2, space="PSUM") as pssm,
            tc.tile_pool(name="psrep", bufs=2, space="PSUM") as psrep,
            tc.tile_pool(name="dramsc", bufs=2, space="DRAM") as dpool,
        ):
            # ---- packed constants + initial tok first (head of pipeline)
            cpack = cpool.tile([128, C_TOT], F32)
            nc.sync.dma_start(cpack[:], cpackh[:])
            tok0 = cpool.tile([128, N], MV, tag="tok0")
            nc.sync.dma_start(tok0[:], tok0h[:])
            ident = cpack[:, C_IDENT : C_IDENT + 128]
            b1c = cpack[:, C_B1 : C_B1 + 1]
            b2c = cpack[:, C_B2 : C_B2 + 1]
            cbg = cpack[:, C_CBG : C_CBG + 1]
            basec = cpack[:, C_BASE : C_BASE + 1]
            baseB = cpack[:, C_BASEB : C_BASEB + N]

            ident_mv = cpool.tile([128, 128], MV, tag="identmv")
            nc.vector.tensor_copy(ident_mv[:], ident)

            # ---- dep replication: chunks 0-1 via DMA broadcast (interleaved
            # with W1), chunks 2-3 via the otherwise-idle gpsimd engine
            repdep = wpool.tile([128, E * N], MV, tag="repdep")
            w1 = wpool.tile([128, (E * D // 128) * T], MV, tag="w1")
            for ch in range(2):
                sl = slice(ch * CH_Z * N, (ch + 1) * CH_Z * N)
                nc.sync.dma_start(
                    repdep[:, sl], deph[0:1, sl].to_broadcast((128, CH_Z * N))
                )
                slw = slice(ch * 16 * 128, (ch + 1) * 16 * 128)
                nc.sync.dma_start(w1[:, slw], w1t[:, slw])
            for ch in range(2, 4):
                drow = rowpool.tile([1, CH_Z * N], MV, tag="drow")
                nc.sync.dma_start(
                    drow[:], deph[0:1, ch * CH_Z * N : (ch + 1) * CH_Z * N]
                )
                nc.gpsimd.partition_broadcast(
                    repdep[:, ch * CH_Z * N : (ch + 1) * CH_Z * N], drow[:]
                )
                slw = slice(ch * 16 * 128, (ch + 1) * 16 * 128)
                nc.sync.dma_start(w1[:, slw], w1t[:, slw])
            # host-built scatter matrices (wr folded; block-1 also mask+cbg)
            Hp = cpool.tile([128, DEPTH * BL * 128], MV, tag="Hp")
            nc.sync.dma_start(Hp[:], hh[:])
            # W2 is only needed at bilinear 2; lands while bilinear 1 runs
            w2 = wpool.tile([128, (T * D // 128) * D], MV, tag="w2")
            for ch in range(8):
                sl = slice(ch * 16 * 128, (ch + 1) * 16 * 128)
                nc.sync.dma_start(w2[:, sl], w2t[:, sl])

            tok_cur = tok0
            for blk in range(DEPTH):
                # ================= bilinear 1 =================
                ps1 = pspool.tile([128, N], F32, tag="psmm")
                for ch in range(NZ):
                    zc = zpool.tile([128, CH_Z * N], MV, tag="zc")
                    nc.vector.tensor_tensor(
                        out=zc[:].rearrange("p (c n) -> p c n", c=CH_Z),
                        in0=tok_cur[:, None, :].to_broadcast((128, CH_Z, N)),
                        in1=repdep[:, ch * CH_Z * N : (ch + 1) * CH_Z * N].rearrange(
                            "p (c n) -> p c n", c=CH_Z
                        ),
                        op=mybir.AluOpType.mult,
                    )
                    for jl in range(CH_Z):
                        i = ch * CH_Z + jl
                        nc.tensor.matmul(
                            ps1[:],
                            lhsT=w1[:, i * 128 : (i + 1) * 128],
                            rhs=zc[:, jl * N : (jl + 1) * N],
                            start=(i == 0),
                            stop=(i == E - 1),
                        )
                tde = work.tile([128, N], MV, tag="tde")
                nc.scalar.activation(
                    tde[:], ps1[:], mybir.ActivationFunctionType.Tanh, bias=b1c
                )

                # spill tde to DRAM for the DMA-broadcast / row-source routes
                route = REP_ROUTES[blk]
                scr = dpool.tile([128, N], MV, tag="scr")
                if "d" in route or "p" in route:
                    nc.sync.dma_start(scr[:], tde[:])
                scr_flat = scr[:].rearrange("j n -> (j n)")

                # ================= bilinear 2 =================
                ps2 = pspool.tile([128, N], F32, tag="psmm")

                def mm2(j, xcbuf, jb):
                    nc.tensor.matmul(
                        ps2[:],
                        lhsT=w2[:, j * 128 : (j + 1) * 128],
                        rhs=xcbuf[:, jb * N : (jb + 1) * N],
                        start=(j == 0),
                        stop=(j == T - 1),
                    )

                for ch in range(NX):
                    r = route[ch % len(route)]
                    j0 = ch * CH_X
                    if r == "e":
                        # PE selector replication, pipelined per 2-j piece:
                        # rep mm x2 -> ACT evac -> DVE xc -> main mm x2
                        for sub in range(CH_X // 2):
                            psb = psrep.tile([128, 2 * N], F32, tag="psrep")
                            for jj in range(2):
                                j = j0 + sub * 2 + jj
                                nc.tensor.matmul(
                                    psb[:, jj * N : (jj + 1) * N],
                                    lhsT=ident_mv[:, j : j + 1].to_broadcast(
                                        (128, 128)
                                    ),
                                    rhs=tde[:, :],
                                    start=True,
                                    stop=True,
                                    skip_group_check=True,
                                )
                            rte = rtpool.tile([128, 2 * N], MV, tag="rte")
                            nc.scalar.activation(
                                rte[:], psb[:], mybir.ActivationFunctionType.Identity
                            )
                            xce = xpool.tile([128, 2 * N], MV, tag="xce")
                            nc.vector.tensor_tensor(
                                out=xce[:].rearrange("p (c n) -> p c n", c=2),
                                in0=tok_cur[:, None, :].to_broadcast((128, 2, N)),
                                in1=rte[:].rearrange("p (c n) -> p c n", c=2),
                                op=mybir.AluOpType.mult,
                            )
                            for jj in range(2):
                                mm2(j0 + sub * 2 + jj, xce, jj)
                        continue
                    if r == "p":
                        # gpsimd broadcast in two 8-j halves for lower latency
                        rt = rtpool.tile([128, CH_X * N], MV, tag="rt")
                        for h in range(2):
                            hw = CH_X // 2 * N
                            row = rowpool.tile([1, hw], MV, tag="row")
                            nc.sync.dma_start(
                                row[:],
                                scr_flat[j0 * N + h * hw : j0 * N + (h + 1) * hw][
                                    None, :
                                ],
                            )
                            nc.gpsimd.partition_broadcast(
                                rt[:, h * hw : (h + 1) * hw], row[:]
                            )
                    else:
                        rt = rtpool.tile([128, CH_X * N], MV, tag="rt")
                        nc.sync.dma_start(
                            rt[:],
                            scr_flat[j0 * N : (j0 + CH_X) * N][None, :].to_broadcast(
                                (128, CH_X * N)
                            ),
                        )
                    xc = xpool.tile([128, CH_X * N], MV, tag="xc")
                    nc.vector.tensor_tensor(
                        out=xc[:].rearrange("p (c n) -> p c n", c=CH_X),
                        in0=tok_cur[:, None, :].to_broadcast((128, CH_X, N)),
                        in1=rt[:].rearrange("p (c n) -> p c n", c=CH_X),
                        op=mybir.AluOpType.mult,
                    )
                    for jl in range(CH_X):
                        mm2(j0 + jl, xc, jl)

                cnz = work.tile([128, N], F32, tag="cnz")
                nc.scalar.activation(
                    cnz[:], ps2[:], mybir.ActivationFunctionType.Tanh, bias=b2c
                )
                last = blk == DEPTH - 1
                if not last:
                    delta = work.tile([128, N], F32, tag="delta")
                    nc.vector.tensor_scalar(
                        out=delta[:],
                        in0=cnz[:],
                        scalar1=cbg,
                        scalar2=None,
                        op0=mybir.AluOpType.subtract,
                    )
                else:
                    # cbg correction is folded into baseB on the host
                    delta = cnz

                # ============ scatter (segment-sum over heads) ============
                if not last:
                    tok_next = work.tile([128, N], MV, tag="tokcur")
                for b in range(BL):
                    psT = pssm.tile([128, 128], F32, tag="pstr")
                    nc.tensor.transpose(
                        psT[:], delta[:, b * 128 : (b + 1) * 128], ident
                    )
                    dT = work.tile([128, 128], MV, tag="dT")
                    nc.vector.tensor_copy(dT[:], psT[:])
                    psS = pssm.tile([128, 128], F32, tag="psS")
                    hcol = (blk * BL + b) * 128
                    if not last:
                        # scat[p,i] = sum_j dT[j,p]^T... = delta @ H
                        nc.tensor.matmul(
                            psS[:],
                            lhsT=dT[:],
                            rhs=Hp[:, hcol : hcol + 128],
                            start=True,
                            stop=True,
                        )
                        nc.scalar.activation(
                            tok_next[:, b * 128 : (b + 1) * 128],
                            psS[:],
                            mybir.ActivationFunctionType.Identity,
                            bias=basec,
                        )
                    else:
                        # transposed scatter: out[i,p] = sum_j H[j,i]*dT[j,p];
                        # mask folded into H, mask*base added via baseB
                        nc.tensor.matmul(
                            psS[:],
                            lhsT=Hp[:, hcol : hcol + 128],
                            rhs=dT[:],
                            start=True,
                            stop=True,
                        )
                        osb = work.tile([128, 128], F32, tag="osb")
                        nc.vector.tensor_tensor(
                            out=osb[:],
                            in0=psS[:],
                            in1=baseB[:, b * 128 : (b + 1) * 128],
                            op=mybir.AluOpType.add,
                        )
                        nc.sync.dma_start(outh[b], osb[:])
                if not last:
                    tok_cur = tok_next
    nc.compile()
    return nc


_NC_CACHE = None


def _get_program():
    global _NC_CACHE
    if _NC_CACHE is None:
        _NC_CACHE = build_program()
    return _NC_CACHE


def kernel(
    token_table,
    dep_table,
    W1,
    b1,
    W2,
    b2,
    wr,
    br,
    tokens,
    dep_types,
    dep_heads,
):
    global LAST_EXEC_TIME_NS
    token_table = np.asarray(token_table, dtype=np.float32)
    dep_table = np.asarray(dep_table, dtype=np.float32)
    W1 = np.asarray(W1, dtype=np.float32)
    b1 = np.asarray(b1, dtype=np.float32)
    W2 = np.asarray(W2, dtype=np.float32)
    b2 = np.asarray(b2, dtype=np.float32)
    wr = np.asarray(wr, dtype=np.float32)
    br = np.asarray(br, dtype=np.float32)
    tokens = np.asarray(tokens).astype(np.int32)
    dep_types = np.asarray(dep_types).astype(np.int32)
    dep_heads = np.asarray(dep_heads).astype(np.int32)

    # weight-layout prep (host): K-tiled stationary operands
    W1f = W1.transpose(2, 1, 0).reshape(E * D, T)  # [(e,d), t]
    W1t = np.ascontiguousarray(
        W1f.reshape(E * D // 128, 128, T).transpose(1, 0, 2).reshape(128, -1)
    ).astype(ml_dtypes.bfloat16)
    W2f = W2.transpose(2, 1, 0).reshape(T * D, D)  # [(t,d), p]
    W2t = np.ascontiguousarray(
        W2f.reshape(T * D // 128, 128, D).transpose(1, 0, 2).reshape(128, -1)
    ).astype(ml_dtypes.bfloat16)
    c_bg = np.tanh(b2)
    base = (np.sum(wr) * c_bg + br).astype(np.float32)

    nc = _get_program()
    in_maps = []
    for c in range(NCORES):
        bs = slice(c * BL, (c + 1) * BL)
        dep_c = dep_table[dep_types[bs]]  # [BL, S, E]
        dep_flat = np.ascontiguousarray(dep_c.reshape(N, E).T.reshape(1, E * N)).astype(
            ml_dtypes.bfloat16
        )
        tok0 = np.ascontiguousarray(
            token_table[tokens[bs]].reshape(N, D).T
        ).astype(ml_dtypes.bfloat16)
        heads_c = dep_heads[bs]
        mask_c = (heads_c == 0).astype(np.float32)  # [BL, S]
        cpack = np.zeros((128, C_TOT), dtype=np.float32)
        cpack[:, C_IDENT : C_IDENT + 128] = np.eye(128, dtype=np.float32)
        cpack[:, C_B1] = b1
        cpack[:, C_B2] = b2
        cpack[:, C_CBG] = c_bg
        cpack[:, C_BASE] = base
        for b in range(BL):
            # baseB[n, d] = mask[b,n] * (base[d] - colsumH[b,n]*cbg[d]);
            # the -cbg part compensates using cnz (not cnz-cbg) in the final
            # transposed scatter matmul
            colsum = np.bincount(heads_c[b], weights=wr, minlength=128)[:128]
            cpack[:, C_BASEB + b * 128 : C_BASEB + (b + 1) * 128] = mask_c[b][
                :, None
            ] * (base[None, :] - np.outer(colsum, c_bg))
        # H[j, (blk,b)*128+i] = wr[j] * (heads[b,j] == i) (* mask for last blk)
        Hpack = np.zeros((128, DEPTH * BL * 128), dtype=np.float32)
        for blk in range(DEPTH):
            for b in range(BL):
                col = (blk * BL + b) * 128
                Hpack[np.arange(S), col + heads_c[b]] = wr
                if blk == DEPTH - 1:
                    Hpack[:, col : col + 128] *= mask_c[b][None, :]
        in_maps.append(
            {
                "W1t": W1t,
                "W2t": W2t,
                "cpack": cpack,
                "Hpack": np.ascontiguousarray(Hpack.astype(ml_dtypes.bfloat16)),
                "tok0": tok0,
                "dep_flat": dep_flat,
            }
        )

    trace = bool(int(os.environ.get("KERNEL_TRACE", "0")))
    res = run_bass_kernel_spmd(nc, in_maps, list(range(NCORES)), trace=trace)
    LAST_EXEC_TIME_NS = res.exec_time_ns
    out = np.concatenate([res.results[c]["out"] for c in range(NCORES)], axis=0)
    return np.ascontiguousarray(out.astype(np.float32))


# revision 16
# speedup vs baseline: 1.0808x; 1.0808x over previous
"""Trainium2 Bass kernel for nn_Composer (gnn_message_passing).

Math per block (DEPTH=2 blocks, same weights):
    tde[t,n]  = tanh( sum_{e,d} W1[t,d,e] * tok[d,n] * dep[e,n] + b1[t] )
    cnz[p,n]  = tanh( sum_{t,d} W2[p,d,t] * tok[d,n] * tde[t,n] + b2[p] )
    tok'[p,i] = base[p] + sum_j wr[j] * (cnz[p,j] - tanh(b2)[p]) * [heads[j]==i]
Final: out = tok * (heads == 0).

Device strategy (8 cores, data-parallel over batch, 2 batches/core, n=256):
  - feature-major layout [feature_partition, n_free]; bf16 pipe (fp32 psum)
  - bilinear contractions as PE matmuls over K-tiles with PSUM accumulation;
    moving operand z = tok * rep(second_factor) built by DVE tensor_tensor
  - the partition-replication of tde is spread across three engines so it
    never serializes on one resource, with a per-block route mix matched to
    when the DMA bus is free:
      * PE one-hot-selector matmul (out[p,n]=sum_k ident[k,j]tde[k,n]) with
        ACT-engine PSUM evacuation -- no DRAM round trip, finest pipelining
      * GPSIMD partition_broadcast from a single-partition SBUF row
      * DMA broadcast from a DRAM scratch copy
  - embeddings (token+dep) gathered on host, shipped in device layout
  - one-hot scatter matrices H built on host, wr folded in; for the final
    block the scatter matmul is emitted TRANSPOSED (lhsT=H, rhs=delta^T)
    so it directly produces the [token, feature] output layout, with the
    root mask and base vector folded in on host
"""

import os
import sys

sys.path.insert(0, "/opt/trn_rl_repo")

import ml_dtypes
import numpy as np

import concourse.bass as bass
import concourse.bacc as bacc
import concourse.mybir as mybir
import concourse.tile as tile
from concourse.bass_utils import run_bass_kernel_spmd

B, S, D, E, T = 16, 128, 128, 64, 128
V_TOK, V_DEP = 100000, 64
DEPTH = 2
NCORES = 8
BL = B // NCORES  # local batches per core
N = BL * S        # positions per core
F32 = mybir.dt.float32
I32 = mybir.dt.int32
BF16 = mybir.dt.bfloat16

CH_Z = 16  # dep-rep chunk size (j-tiles per chunk; 64 z-tiles total)
CH_X = 16  # tde-rep chunk size (128 x-tiles total)

# bilinear-2 replication route per 16-j chunk, per block:
#   'e' = PE selector-matmul, 'p' = gpsimd partition_broadcast, 'd' = DMA
REP_ROUTES = [
    ["e", "p", "d", "e", "p", "d", "e", "d"],  # block 0: DMA busy with W2
    ["e", "d", "d", "p", "d", "d", "p", "d"],  # block 1: DMA bus is free
]

# packed f32 constant layout (columns)
C_IDENT = 0          # [0,128)   identity
C_B1 = 128           # b1
C_B2 = 129           # b2
C_CBG = 130          # tanh(b2)
C_BASE = 131         # sum(wr)*tanh(b2)+br
C_BASEB = 132        # [132,132+N): outer(mask_b, base) per batch, [n,d] rows
C_TOT = 132 + N

LAST_EXEC_TIME_NS = None


def build_program():
    MV = BF16
    nc = bacc.Bacc("TRN2", target_bir_lowering=False, debug=False)
    w1t = nc.dram_tensor("W1t", [128, (E * D // 128) * T], MV, kind="ExternalInput")
    w2t = nc.dram_tensor("W2t", [128, (T * D // 128) * D], MV, kind="ExternalInput")
    cpackh = nc.dram_tensor("cpack", [128, C_TOT], F32, kind="ExternalInput")
    hh = nc.dram_tensor("Hpack", [128, DEPTH * BL * 128], MV, kind="ExternalInput")
    tok0h = nc.dram_tensor("tok0", [128, N], MV, kind="ExternalInput")
    deph = nc.dram_tensor("dep_flat", [1, E * N], MV, kind="ExternalInput")
    outh = nc.dram_tensor("out", [BL, S, D], F32, kind="ExternalOutput")

    NZ = E // CH_Z
    NX = T // CH_X

    with tile.TileContext(nc) as tc:
        with (
            tc.tile_pool(name="const", bufs=1) as cpool,
            tc.tile_pool(name="wres", bufs=1) as wpool,
            tc.tile_pool(name="zc", bufs=2) as zpool,
            tc.tile_pool(name="rept", bufs=3) as rtpool,
            tc.tile_pool(name="reprow", bufs=2) as rowpool,
            tc.tile_pool(name="xc", bufs=3) as xpool,
            tc.tile_pool(name="work", bufs=2) as work,
            tc.tile_pool(name="psmm", bufs=2, space="PSUM") as pspool,
            tc.tile_pool(name="pssm", bufs=2, space="PSUM") as pssm,
            tc.tile_pool(name="psrep", bufs=2, space="PSUM") as psrep,
            tc.tile_pool(name="dramsc", bufs=2, space="DRAM") as dpool,
        ):
            # ---- packed constants + initial tok first (head of pipeline)
            cpack = cpool.tile([128, C_TOT], F32)
            nc.sync.dma_start(cpack[:], cpackh[:])
            tok0 = cpool.tile([128, N], MV, tag="tok0")
            nc.sync.dma_start(tok0[:], tok0h[:])
            ident = cpack[:, C_IDENT : C_IDENT + 128]
            b1c = cpack[:, C_B1 : C_B1 + 1]
            b2c = cpack[:, C_B2 : C_B2 + 1]
            cbg = cpack[:, C_CBG : C_CBG + 1]
            basec = cpack[:, C_BASE : C_BASE + 1]
            baseB = cpack[:, C_BASEB : C_BASEB + N]

            ident_mv = cpool.tile([128, 128], MV, tag="identmv")
            nc.vector.tensor_copy(ident_mv[:], ident)

            # ---- dep replication: chunks 0-1 via DMA broadcast (interleaved
            # with W1), chunks 2-3 via the otherwise-idle gpsimd engine
            repdep = wpool.tile([128, E * N], MV, tag="repdep")
            w1 = wpool.tile([128, (E * D // 128) * T], MV, tag="w1")
            for ch in range(2):
                sl = slice(ch * CH_Z * N, (ch + 1) * CH_Z * N)
                nc.sync.dma_start(
                    repdep[:, sl], deph[0:1, sl].to_broadcast((128, CH_Z * N))
                )
                slw = slice(ch * 16 * 128, (ch + 1) * 16 * 128)
                nc.sync.dma_start(w1[:, slw], w1t[:, slw])
            for ch in range(2, 4):
                drow = rowpool.tile([1, CH_Z * N], MV, tag="drow")
                nc.sync.dma_start(
                    drow[:], deph[0:1, ch * CH_Z * N : (ch + 1) * CH_Z * N]
                )
                nc.gpsimd.partition_broadcast(
                    repdep[:, ch * CH_Z * N : (ch + 1) * CH_Z * N], drow[:]
                )
                slw = slice(ch * 16 * 128, (ch + 1) * 16 * 128)
                nc.sync.dma_start(w1[:, slw], w1t[:, slw])
            # host-built scatter matrices (wr folded; block-1 also mask+cbg)
            Hp = cpool.tile([128, DEPTH * BL * 128], MV, tag="Hp")
            nc.sync.dma_start(Hp[:], hh[:])
            # W2 is only needed at bilinear 2; lands while bilinear 1 runs
            w2 = wpool.tile([128, (T * D // 128) * D], MV, tag="w2")
            for ch in range(8):
                sl = slice(ch * 16 * 128, (ch + 1) * 16 * 128)
                nc.sync.dma_start(w2[:, sl], w2t[:, sl])

            tok_cur = tok0
            for blk in range(DEPTH):
                # ================= bilinear 1 =================
                ps1 = pspool.tile([128, N], F32, tag="psmm")
                for ch in range(NZ):
                    zc = zpool.tile([128, CH_Z * N], MV, tag="zc")
                    nc.vector.tensor_tensor(
                        out=zc[:].rearrange("p (c n) -> p c n", c=CH_Z),
                        in0=tok_cur[:, None, :].to_broadcast((128, CH_Z, N)),
                        in1=repdep[:, ch * CH_Z * N : (ch + 1) * CH_Z * N].rearrange(
                            "p (c n) -> p c n", c=CH_Z
                        ),
                        op=mybir.AluOpType.mult,
                    )
                    for jl in range(CH_Z):
                        i = ch * CH_Z + jl
                        nc.tensor.matmul(
                            ps1[:],
                            lhsT=w1[:, i * 128 : (i + 1) * 128],
                            rhs=zc[:, jl * N : (jl + 1) * N],
                            start=(i == 0),
                            stop=(i == E - 1),
                        )
                tde = work.tile([128, N], MV, tag="tde")
                nc.scalar.activation(
                    tde[:], ps1[:], mybir.ActivationFunctionType.Tanh, bias=b1c
                )

                # spill tde to DRAM for the DMA-broadcast / row-source routes
                route = REP_ROUTES[blk]
                scr = dpool.tile([128, N], MV, tag="scr")
                if "d" in route or "p" in route:
                    nc.sync.dma_start(scr[:], tde[:])
                scr_flat = scr[:].rearrange("j n -> (j n)")

                # ================= bilinear 2 =================
                ps2 = pspool.tile([128, N], F32, tag="psmm")

                def mm2(j, xcbuf, jb):
                    nc.tensor.matmul(
                        ps2[:],
                        lhsT=w2[:, j * 128 : (j + 1) * 128],
                        rhs=xcbuf[:, jb * N : (jb + 1) * N],
                        start=(j == 0),
                        stop=(j == T - 1),
                    )

                for ch in range(NX):
                    r = route[ch % len(route)]
                    j0 = ch * CH_X
                    if r == "e":
                        # PE selector replication, pipelined per 2-j piece:
                        # rep mm x2 -> ACT evac -> DVE xc -> main mm x2
                        for sub in range(CH_X // 2):
                            psb = psrep.tile([128, 2 * N], F32, tag="psrep")
                            for jj in range(2):
                                j = j0 + sub * 2 + jj
                                nc.tensor.matmul(
                                    psb[:, jj * N : (jj + 1) * N],
                                    lhsT=ident_mv[:, j : j + 1].to_broadcast(
                                        (128, 128)
                                    ),
                                    rhs=tde[:, :],
                                    start=True,
                                    stop=True,
                                    skip_group_check=True,
                                )
                            rte = rtpool.tile([128, 2 * N], MV, tag="rte")
                            nc.scalar.activation(
                                rte[:], psb[:], mybir.ActivationFunctionType.Identity
                            )
                            xce = xpool.tile([128, 2 * N], MV, tag="xce")
                            nc.vector.tensor_tensor(
                                out=xce[:].rearrange("p (c n) -> p c n", c=2),
                                in0=tok_cur[:, None, :].to_broadcast((128, 2, N)),
                                in1=rte[:].rearrange("p (c n) -> p c n", c=2),
                                op=mybir.AluOpType.mult,
                            )
                            for jj in range(2):
                                mm2(j0 + sub * 2 + jj, xce, jj)
                        continue
                    if r == "p":
                        # gpsimd broadcast in two 8-j halves for lower latency
                        rt = rtpool.tile([128, CH_X * N], MV, tag="rt")
                        for h in range(2):
                            hw = CH_X // 2 * N
                            row = rowpool.tile([1, hw], MV, tag="row")
                            nc.sync.dma_start(
                                row[:],
                                scr_flat[j0 * N + h * hw : j0 * N + (h + 1) * hw][
                                    None, :
                                ],
                            )
                            nc.gpsimd.partition_broadcast(
                                rt[:, h * hw : (h + 1) * hw], row[:]
                            )
                    else:
                        rt = rtpool.tile([128, CH_X * N], MV, tag="rt")
                        nc.sync.dma_start(
                            rt[:],
                            scr_flat[j0 * N : (j0 + CH_X) * N][None, :].to_broadcast(
                                (128, CH_X * N)
                            ),
                        )
                    xc = xpool.tile([128, CH_X * N], MV, tag="xc")
                    nc.vector.tensor_tensor(
                        out=xc[:].rearrange("p (c n) -> p c n", c=CH_X),
                        in0=tok_cur[:, None, :].to_broadcast((128, CH_X, N)),
                        in1=rt[:].rearrange("p (c n) -> p c n", c=CH_X),
                        op=mybir.AluOpType.mult,
                    )
                    for jl in range(CH_X):
                        mm2(j0 + jl, xc, jl)

                cnz = work.tile([128, N], F32, tag="cnz")
                nc.scalar.activation(
                    cnz[:], ps2[:], mybir.ActivationFunctionType.Tanh, bias=b2c
                )
                last = blk == DEPTH - 1
                if not last:
                    delta = work.tile([128, N], F32, tag="delta")
                    nc.vector.tensor_scalar(
                        out=delta[:],
                        in0=cnz[:],
                        scalar1=cbg,
                        scalar2=None,
                        op0=mybir.AluOpType.subtract,
                    )
                else:
                    # cbg correction is folded into baseB on the host
                    delta = cnz

                # ============ scatter (segment-sum over heads) ============
                if not last:
                    tok_next = work.tile([128, N], MV, tag="tokcur")
                for b in range(BL):
                    psT = pssm.tile([128, 128], F32, tag="pstr")
                    nc.tensor.transpose(
                        psT[:], delta[:, b * 128 : (b + 1) * 128], ident
                    )
                    dT = work.tile([128, 128], MV, tag="dT")
                    nc.vector.tensor_copy(dT[:], psT[:])
                    psS = pssm.tile([128, 128], F32, tag="psS")
                    hcol = (blk * BL + b) * 128
                    if not last:
                        # scat[p,i] = sum_j dT[j,p]^T... = delta @ H
                        nc.tensor.matmul(
                            psS[:],
                            lhsT=dT[:],
                            rhs=Hp[:, hcol : hcol + 128],
                            start=True,
                            stop=True,
                        )
                        nc.scalar.activation(
                            tok_next[:, b * 128 : (b + 1) * 128],
                            psS[:],
                            mybir.ActivationFunctionType.Identity,
                            bias=basec,
                        )
                    else:
                        # transposed scatter: out[i,p] = sum_j H[j,i]*dT[j,p];
                        # mask folded into H, mask*base added via baseB
                        nc.tensor.matmul(
                            psS[:],
                            lhsT=Hp[:, hcol : hcol + 128],
                            rhs=dT[:],
                            start=True,
                            stop=True,
                        )
                        osb = work.tile([128, 128], F32, tag="osb")
                        nc.vector.tensor_tensor(
                            out=osb[:],
                            in0=psS[:],
                            in1=baseB[:, b * 128 : (b + 1) * 128],
                            op=mybir.AluOpType.add,
                        )
                        nc.sync.dma_start(outh[b], osb[:])
                if not last:
                    tok_cur = tok_next
    nc.compile()
    return nc


_NC_CACHE = None


def _get_program():
    global _NC_CACHE
    if _NC_CACHE is None:
        _NC_CACHE = build_program()
    return _NC_CACHE


def kernel(
    token_table,
    dep_table,
    W1,
    b1,
    W2,
    b2,
    wr,
    br,
    tokens,
    dep_types,
    dep_heads,
):
    global LAST_EXEC_TIME_NS
    token_table = np.asarray(token_table, dtype=np.float32)
    dep_table = np.asarray(dep_table, dtype=np.float32)
    W1 = np.asarray(W1, dtype=np.float32)
    b1 = np.asarray(b1, dtype=np.float32)
    W2 = np.asarray(W2, dtype=np.float32)
    b2 = np.asarray(b2, dtype=np.float32)
    wr = np.asarray(wr, dtype=np.float32)
    br = np.asarray(br, dtype=np.float32)
    tokens = np.asarray(tokens).astype(np.int32)
    dep_types = np.asarray(dep_types).astype(np.int32)
    dep_heads = np.asarray(dep_heads).astype(np.int32)

    # weight-layout prep (host): K-tiled stationary operands
    W1f = W1.transpose(2, 1, 0).reshape(E * D, T)  # [(e,d), t]
    W1t = np.ascontiguousarray(
        W1f.reshape(E * D // 128, 128, T).transpose(1, 0, 2).reshape(128, -1)
    ).astype(ml_dtypes.bfloat16)
    W2f = W2.transpose(2, 1, 0).reshape(T * D, D)  # [(t,d), p]
    W2t = np.ascontiguousarray(
        W2f.reshape(T * D // 128, 128, D).transpose(1, 0, 2).reshape(128, -1)
    ).astype(ml_dtypes.bfloat16)
    c_bg = np.tanh(b2)
    base = (np.sum(wr) * c_bg + br).astype(np.float32)

    nc = _get_program()
    in_maps = []
    for c in range(NCORES):
        bs = slice(c * BL, (c + 1) * BL)
        dep_c = dep_table[dep_types[bs]]  # [BL, S, E]
        dep_flat = np.ascontiguousarray(dep_c.reshape(N, E).T.reshape(1, E * N)).astype(
            ml_dtypes.bfloat16
        )
        tok0 = np.ascontiguousarray(
            token_table[tokens[bs]].reshape(N, D).T
        ).astype(ml_dtypes.bfloat16)
        heads_c = dep_heads[bs]
        mask_c = (heads_c == 0).astype(np.float32)  # [BL, S]
        cpack = np.zeros((128, C_TOT), dtype=np.float32)
        cpack[:, C_IDENT : C_IDENT + 128] = np.eye(128, dtype=np.float32)
        cpack[:, C_B1] = b1
        cpack[:, C_B2] = b2
        cpack[:, C_CBG] = c_bg
        cpack[:, C_BASE] = base
        for b in range(BL):
            # baseB[n, d] = mask[b,n] * (base[d] - colsumH[b,n]*cbg[d]);
            # the -cbg part compensates using cnz (not cnz-cbg) in the final
            # transposed scatter matmul
            colsum = np.bincount(heads_c[b], weights=wr, minlength=128)[:128]
            cpack[:, C_BASEB + b * 128 : C_BASEB + (b + 1) * 128] = mask_c[b][
                :, None
            ] * (base[None, :] - np.outer(colsum, c_bg))
        # H[j, (blk,b)*128+i] = wr[j] * (heads[b,j] == i) (* mask for last blk)
        Hpack = np.zeros((128, DEPTH * BL * 128), dtype=np.float32)
        for blk in range(DEPTH):
            for b in range(BL):
                col = (blk * BL + b) * 128
                Hpack[np.arange(S), col + heads_c[b]] = wr
                if blk == DEPTH - 1:
                    Hpack[:, col : col + 128] *= mask_c[b][None, :]
        in_maps.append(
            {
                "W1t": W1t,
                "W2t": W2t,
                "cpack": cpack,
                "Hpack": np.ascontiguousarray(Hpack.astype(ml_dtypes.bfloat16)),
                "tok0": tok0,
                "dep_flat": dep_flat,
            }
        )

    trace = bool(int(os.environ.get("KERNEL_TRACE", "0")))
    res = run_bass_kernel_spmd(nc, in_maps, list(range(NCORES)), trace=trace)
    LAST_EXEC_TIME_NS = res.exec_time_ns
    out = np.concatenate([res.results[c]["out"] for c in range(NCORES)], axis=0)
    return np.ascontiguousarray(out.astype(np.float32))
